# revision 45
# baseline (speedup 1.0000x reference)
"""Distributed attention kernel for Trainium2 (8 NeuronCores).

Reference computation (B=2, N=2048, C=1024, H=16, D=64, ALPHA=0.5):
    qkv = x @ W_qkv -> q,k,v [B,H,N,D]
    attn = softmax(q @ k^T / sqrt(D))
    attn = 0.5*dm + 0.5*attn
    out  = (attn @ v).reshape(B,N,C) @ W_proj + b_proj

Sharding: 8 cores = 2 batches x 4 head-groups (4 heads each).
Each core computes its head-group's slice end-to-end, including a partial
projection (row-slice of W_proj); host sums the 4 partials per batch.

On-device layout strategy (per core):
  - x arrives transposed [C, N]; q,k are produced transposed [Dg=256, N]
    (head-dim on partitions); scores are computed transposed
    S^T[m, q] = k^T.T @ q^T so exp runs on ScalarE straight out of PSUM.
  - attn@v runs in the *natural* orientation out[q, d] with the exp tile as
    the stationary operand (lhsT = e^T[m, q-tile 128], rhs = v[m, 65]):
    contraction is the full 128 m-rows AND the output uses all 128 q
    partitions (the transposed form only fills 65 of 128 output rows).
  - v carries an appended column holding 2.0, so out[q, 64] = 2*r_q (the
    softmax denominator); normalization is a per-partition multiply by
    0.5/r_q (vector.reciprocal of the 2r column) fused with the dm@v add
    via scalar_tensor_tensor.
  - dm@v accumulates in the same [q, dg] orientation (lhsT = dm^T tile).
  - The [q, dg] result is transposed back to [dg, q] for the W_proj
    contraction with cheap PE transposes ([128,128] identity matmuls).
  - Schedule: the exp stream on ScalarE (~1.04us per [128,1024] tile, 128
    tiles = 134us) is the secondary critical path after the PE (~168us
    busy), so the first score matmul must issue early and neither engine
    may stall at pass boundaries.  The prologue runs 12 projection groups
    ct-outer while the x tiles stream in (k-jo0 all + q-jo0-nq0 in psS
    slot halves, v m-tiles 0..7 paired in the a/x banks); the remaining
    v/q/k groups, dm@v, and the W_proj groups of the previous q-chunk are
    woven into the per-mt loops of the eight attention passes.  The last
    two e@v emissions and the epilogue of each pass slide into the next
    pass's first iterations ("carry") so the next score stream issues
    immediately.  dm@v bank grabs start at mt>=2 so they never
    head-of-line block scores behind a pending DVE copy.
  - PSUM budget (8 banks): scores [128,1024] x2 bufs = 4; e@v accumulators
    = 2 banks, each holding two q-subtile groups [128,130] at 256-col
    offsets -- only the first matmul per bank uses start=True (hardware
    zeroes the whole 2KB bank region on start), every other group
    accumulates with start=False onto pending-zero bytes; 2 "x" banks
    rotate between prologue groups, dm@v accumulation and W_proj groups;
    transposes ride the psS slots.
  - max-subtraction is skipped: scores are ~N(0,1), exp never overflows.
  - all matmul operands are fp16; PSUM accumulation stays fp32.
"""

import numpy as np

B, N, C, H, D = 2, 2048, 1024, 16, 64
NCORES = 8
HG = 4                # head-groups per batch
HPC = H // HG         # heads per core = 4
DG = HPC * D          # 256: head-group width
SCALE = D ** -0.5

KT = C // 128         # 8 contraction tiles for qkv/x
MT = N // 128         # 16 m (key) tiles
NQ = N // 512         # 4 q-chunks
QT = N // 128         # 16 q-tiles


def _build_program():
    import concourse.bass as bass
    import concourse.bacc as bacc
    import concourse.tile as tile
    from concourse import mybir
    from contextlib import ExitStack

    f32 = mybir.dt.float32
    f16 = mybir.dt.float16
    Exp = mybir.ActivationFunctionType.Exp
    Mult = mybir.AluOpType.mult
    Add = mybir.AluOpType.add

    nc = bacc.Bacc()
    xT = nc.declare_dram_parameter("xT", [C, N], f16, isOutput=False)
    wq = nc.declare_dram_parameter("wq", [128, KT * DG], f16, isOutput=False)
    wk = nc.declare_dram_parameter("wk", [128, KT * DG], f16, isOutput=False)
    wv = nc.declare_dram_parameter("wv", [128, KT * DG], f16, isOutput=False)
    wp = nc.declare_dram_parameter("wp", [128, 2 * C], f16, isOutput=False)
    dmt = nc.declare_dram_parameter("dmt", [128, MT * N], f16, isOutput=False)
    ident = nc.declare_dram_parameter("ident", [128, 128], f16, isOutput=False)
    pout = nc.declare_dram_parameter("pout", [C, N], f16, isOutput=True)

    with tile.TileContext(nc) as tc, ExitStack() as ctx:
        big = ctx.enter_context(tc.tile_pool(name="big", bufs=1))
        epool = ctx.enter_context(tc.tile_pool(name="epool", bufs=8))
        small = ctx.enter_context(tc.tile_pool(name="small", bufs=2))
        outp = ctx.enter_context(tc.tile_pool(name="outp", bufs=4))
        # PSUM: psS 2x[128,1024] = 4 banks, psA 2 banks, psX 2 banks.
        psS = ctx.enter_context(tc.tile_pool(name="psS", bufs=2, space="PSUM"))
        psA = ctx.enter_context(tc.tile_pool(name="psA", bufs=1, space="PSUM"))
        psX = ctx.enter_context(tc.tile_pool(name="psX", bufs=1, space="PSUM"))

        xt = big.tile([128, KT, N], f16)
        wq_s = big.tile([128, KT, DG], f16)
        wk_s = big.tile([128, KT, DG], f16)
        wv_s = big.tile([128, KT, DG], f16)
        wp_s = big.tile([128, 2, C], f16)
        dms = big.tile([128, MT, N], f16)
        qt = big.tile([128, 2, N], f16)
        kt = big.tile([128, 2, N], f16)
        vaug = big.tile([128, MT, HPC, D + 1], f16)
        outacc = big.tile([128, QT, DG], f16)
        dmacc = big.tile([128, QT, DG], f16)
        outT = big.tile([128, 2, N], f16)
        ident_s = big.tile([128, 128], f16)
        ones_sb = big.tile([128, MT * HPC], f32)

        nc.vector.memset(ones_sb[:, :], 2.0)
        nc.vector.tensor_copy(vaug[:, :, :, D], ones_sb[:, :])

        # ---- input DMA: x per-ct (streams the ct-outer prologue), weights
        # one DMA each (host-packed rows), dm in 4 chunks.
        nc.sync.dma_start(out=xt[:, 0, :], in_=xT[0:128, :])
        nc.sync.dma_start(out=wk_s[:, :, :], in_=wk[:, :])
        nc.sync.dma_start(out=wq_s[:, :, :], in_=wq[:, :])
        nc.sync.dma_start(out=wv_s[:, :, :], in_=wv[:, :])
        for ct in range(1, KT):
            nc.sync.dma_start(out=xt[:, ct, :], in_=xT[ct * 128:(ct + 1) * 128, :])
        nc.sync.dma_start(out=ident_s[:, :], in_=ident[:, :])
        nc.sync.dma_start(out=wp_s[:, :, :], in_=wp[:, :])
        for h in range(4):
            nc.sync.dma_start(out=dms[:, 4 * h:4 * h + 4, :],
                              in_=dmt[:, 4 * h * N:(4 * h + 4) * N])

        # ---- prologue: 12 projection groups accumulate ct-outer while the
        # x tiles stream in.  psS slots hold two bank-groups each; the a/x
        # banks hold two v-groups each (single-start-per-bank).
        slotA = psS.tile([128, 1024], f32, name="slotA", tag="psS")
        slotB = psS.tile([128, 1024], f32, name="slotB", tag="psS")
        vslots = {}
        for i, tg in enumerate(("a0", "a1", "x0", "x1")):
            pool = psA if tg.startswith("a") else psX
            vslots[tg] = pool.tile([128, 512], f32, name=f"vs{i}", tag=tg)

        def pro_w(ct, w_s, jo, nqi, dst, first, last):
            nc.tensor.matmul(
                dst, lhsT=w_s[:, ct, jo * 128:(jo + 1) * 128],
                rhs=xt[:, ct, nqi * 512:(nqi + 1) * 512],
                start=first, stop=last, skip_group_check=True)

        def pro_v(ct, mt, first, last):
            bank = vslots[("a0", "a1", "x0", "x1")[mt // 2]]
            nc.tensor.matmul(
                bank[:, (mt % 2) * 256:(mt % 2) * 256 + DG],
                lhsT=xt[:, ct, mt * 128:(mt + 1) * 128],
                rhs=wv_s[:, ct, :],
                start=first and mt % 2 == 0, stop=last,
                skip_group_check=True)

        for ct in range(KT):
            fi, la = ct == 0, ct == KT - 1
            pro_w(ct, wk_s, 0, 0, slotA[:, 0:512], fi, la)
            pro_w(ct, wq_s, 0, 0, slotA[:, 512:1024], fi, la)
            pro_w(ct, wk_s, 0, 1, slotB[:, 0:512], fi, la)
            pro_w(ct, wk_s, 0, 2, slotB[:, 512:1024], fi, la)
            for mt in range(8):
                pro_v(ct, mt, fi, la)
        nc.vector.tensor_copy(kt[:, 0, 0:512], slotA[:, 0:512])
        nc.vector.tensor_scalar_mul(qt[:, 0, 0:512], slotA[:, 512:1024], SCALE)
        nc.vector.tensor_copy(kt[:, 0, 512:1024], slotB[:, 0:512])
        nc.vector.tensor_copy(kt[:, 0, 1024:1536], slotB[:, 512:1024])
        for i, tg in enumerate(("a0", "a1", "x0", "x1")):
            nc.vector.tensor_copy(vaug[:, 2 * i:2 * i + 2, :, 0:D], vslots[tg][:, :])

        # ---- deferred one-time groups, woven into the passes as lumps ----
        def _xtile(tag):
            pool = psA if tag.startswith("a") else (psS if tag == "psS" else psX)
            return pool.tile([128, 512], f32, name=f"lump_{tag}", tag=tag)

        xrot = [0]

        def xtag():
            xrot[0] ^= 1
            return f"x{xrot[0]}"

        def k_group(jo, nqi, tag=None):
            ps = _xtile(tag or xtag())
            for i in range(KT):
                ct = (nqi + i) % KT
                nc.tensor.matmul(
                    ps[:, :],
                    lhsT=wk_s[:, ct, jo * 128:(jo + 1) * 128],
                    rhs=xt[:, ct, nqi * 512:(nqi + 1) * 512],
                    start=(i == 0), stop=(i == KT - 1),
                )
            nc.vector.tensor_copy(kt[:, jo, nqi * 512:(nqi + 1) * 512], ps[:, :])

        def q_group(jo, nqi, tag=None):
            ps = _xtile(tag or xtag())
            for i in range(KT):
                ct = (nqi + i) % KT
                nc.tensor.matmul(
                    ps[:, :],
                    lhsT=wq_s[:, ct, jo * 128:(jo + 1) * 128],
                    rhs=xt[:, ct, nqi * 512:(nqi + 1) * 512],
                    start=(i == 0), stop=(i == KT - 1),
                )
            nc.vector.tensor_scalar_mul(qt[:, jo, nqi * 512:(nqi + 1) * 512], ps[:, :], SCALE)

        def v_group(mt):
            ps = psX.tile([128, DG], f32, name="vps", tag=xtag(),
                          padded_shape=[128, 512])
            for i in range(KT):
                ct = (mt + i) % KT
                nc.tensor.matmul(
                    ps[:, :],
                    lhsT=xt[:, ct, mt * 128:(mt + 1) * 128],
                    rhs=wv_s[:, ct, :],
                    start=(i == 0), stop=(i == KT - 1),
                )
            nc.vector.tensor_copy(vaug[:, mt, :, 0:D], ps[:, :])

        def make_dm_fill(nqi, compact=False):
            state = {}

            def step(mm):
                if not state:
                    state["t"] = [psX.tile([128, 512], f32, name=f"dmps{i}", tag=f"x{i}")
                                  for i in range(2)]
                for qs in range(4):
                    qti = nqi * 4 + qs
                    bank = state["t"][qs // 2]
                    base = (qs % 2) * 256
                    nc.tensor.matmul(
                        bank[:, base:base + DG],
                        lhsT=dms[:, mm, qti * 128:(qti + 1) * 128],
                        rhs=vaug[:, mm, :, 0:D],
                        start=(mm == 0 and qs % 2 == 0),
                        stop=(mm == MT - 1 and qs % 2 == 1),
                        skip_group_check=True,
                    )

            def fill(mt):
                if compact:
                    # 16 steps over mt 10..15 (the x banks host one-time k/q
                    # groups earlier in this pass)
                    sched = {10: (0, 3), 11: (3, 6), 12: (6, 9),
                             13: (9, 12), 14: (12, 14), 15: (14, 16)}
                    if mt in sched:
                        for s in range(*sched[mt]):
                            step(s)
                else:
                    # start at mt 2 so the bank grab never head-of-line
                    # blocks the first score matmuls of the pass
                    if 2 <= mt <= 13:
                        step(mt - 2)
                    elif mt == 14:
                        step(12), step(13)
                    elif mt == 15:
                        step(14), step(15)

            def finish():
                for i in range(2):
                    q0 = nqi * 4 + 2 * i
                    nc.vector.tensor_copy(dmacc[:, q0:q0 + 2, :], state["t"][i][:, :])

            return fill, finish

        def proj_group(nqi, co, tags=("x0", "x1"), act_copy=False):
            qsl = slice(nqi * 512, (nqi + 1) * 512)
            tg = tags[co % len(tags)]
            pool = psA if tg.startswith("a") else psX
            ps = pool.tile([128, 512], f32, name="pps", tag=tg)
            for jo in range(2):
                nc.tensor.matmul(
                    ps[:, :],
                    lhsT=wp_s[:, jo, co * 128:(co + 1) * 128],
                    rhs=outT[:, jo, qsl],
                    start=(jo == 0), stop=(jo == 1),
                )
            so = outp.tile([128, 512], f16, name="so")
            if act_copy:
                nc.scalar.copy(so[:, :], ps[:, :])
            else:
                nc.vector.tensor_copy(so[:, :], ps[:, :])
            nc.sync.dma_start(out=pout[co * 128:(co + 1) * 128, qsl], in_=so[:, :])

        def transposes(nqi, jo):
            # via psS slots (the x banks hold persistent dm accumulators)
            for qs in range(4):
                qti = nqi * 4 + qs
                tr = psS.tile([128, 128], f16, name="tr", tag="psS",
                              padded_shape=[128, 512])
                nc.tensor.transpose(tr[:, :], outacc[:, qti, jo * 128:(jo + 1) * 128],
                                    ident_s[:, :])
                nc.vector.tensor_copy(outT[:, jo, qti * 128:(qti + 1) * 128], tr[:, :])

        # ---- attention pass: scores + exp + e@v for one head pair / q-chunk
        def emit_eav(nqi, hp, eav, mt, et):
            for qs in range(4):
                bank = eav[qs // 2]
                base = (qs % 2) * 256
                for h2 in range(2):
                    nc.tensor.matmul(
                        bank[:, base + h2 * 65: base + h2 * 65 + 65],
                        lhsT=et[:, h2 * 512 + qs * 128: h2 * 512 + (qs + 1) * 128],
                        rhs=vaug[:, mt, 2 * hp + h2, :],
                        start=(mt == 0 and qs % 2 == 0 and h2 == 0),
                        stop=(mt == MT - 1 and qs % 2 == 1 and h2 == 1),
                        skip_group_check=True,
                    )

        # carry: the previous pass's last two e@v emissions and its epilogue
        # slide into the next pass's first iterations, so the next score
        # stream issues immediately and ScalarE never idles at a boundary.
        carry = {}

        def attn_pass(nqi, hp, fill=None, lumps=None, post=(), defer=3):
            qsl = slice(nqi * 512, (nqi + 1) * 512)
            eav = [psA.tile([128, 512], f32, name=f"eav{i}", tag=f"a{i}")
                   for i in range(2)] if not carry else None
            pend = []
            prev = dict(carry) if carry else None
            carry.clear()
            post = list(post)
            for mt in range(MT):
                if lumps and mt in lumps:
                    for th in lumps[mt]:
                        th()
                if fill is not None:
                    fill(mt)
                msl = slice(mt * 128, (mt + 1) * 128)
                sps = psS.tile([128, 1024], f32, name="sps", tag="psS")
                nc.tensor.matmul(sps[:, 0:512], lhsT=kt[0:D, hp, msl],
                                 rhs=qt[0:D, hp, qsl], start=True, stop=True)
                nc.tensor.matmul(sps[:, 512:1024], lhsT=kt[D:128, hp, msl],
                                 rhs=qt[D:128, hp, qsl], start=True, stop=True)
                et = epool.tile([128, 1024], f16, name="et", tag="et")
                nc.scalar.activation(et[:, :], sps[:, :], Exp)
                pend.append((mt, et))
                if prev is not None:
                    if prev["pend"]:
                        emit_eav(prev["nqi"], prev["hp"], prev["eav"],
                                 *prev["pend"].pop(0))
                    if not prev["pend"]:
                        for th in post:
                            th()
                        post = []
                        prev = None
                        eav = [psA.tile([128, 512], f32, name=f"eav{i}", tag=f"a{i}")
                               for i in range(2)]
                elif len(pend) > defer:
                    emit_eav(nqi, hp, eav, *pend.pop(0))
            while len(pend) > 2:
                emit_eav(nqi, hp, eav, *pend.pop(0))
            carry.update(dict(nqi=nqi, hp=hp, eav=eav, pend=pend))
            return eav

        def flush_carry():
            prev = dict(carry)
            carry.clear()
            while prev["pend"]:
                emit_eav(prev["nqi"], prev["hp"], prev["eav"], *prev["pend"].pop(0))
            return prev["eav"]

        def epilogue(nqi, hp, eav, with_dm, qs_list=range(4)):
            for qs in qs_list:
                qti = nqi * 4 + qs
                bank = eav[qs // 2]
                base = (qs % 2) * 256
                rec = small.tile([128, 2], f32, name="rec", tag="rec")
                with nc.allow_low_precision(reason="0.5/r per-q reciprocal"):
                    for h2 in range(2):
                        nc.vector.reciprocal(rec[:, h2:h2 + 1],
                                             bank[:, base + h2 * 65 + 64: base + h2 * 65 + 65])
                for h2 in range(2):
                    col = base + h2 * 65
                    dst = outacc[:, qti, (2 * hp + h2) * 64:(2 * hp + h2 + 1) * 64]
                    if with_dm:
                        nc.vector.scalar_tensor_tensor(
                            dst, bank[:, col:col + 64], rec[:, h2:h2 + 1],
                            dmacc[:, qti, (2 * hp + h2) * 64:(2 * hp + h2 + 1) * 64],
                            op0=Mult, op1=Add)
                    else:
                        nc.vector.tensor_scalar_mul(dst, bank[:, col:col + 64],
                                                    rec[:, h2:h2 + 1])

        # ---- main schedule ----
        L = lambda f, *a, **k: (lambda: f(*a, **k))
        lumps00 = {
            3: [L(v_group, 8)], 4: [L(v_group, 9)],
            5: [L(k_group, 0, 3)], 6: [L(k_group, 1, 0)],
            7: [L(v_group, 10)], 8: [L(q_group, 1, 0)],
            9: [L(v_group, 11)], 10: [L(v_group, 12)],
            11: [L(v_group, 13)], 12: [L(v_group, 14)],
            13: [L(v_group, 15)],
        }
        eav00 = attn_pass(0, 0, lumps=lumps00)
        dmfill, dmfin0 = make_dm_fill(0, compact=True)
        lumps01 = {1: [L(k_group, 1, 1)], 3: [L(k_group, 1, 2)],
                   5: [L(k_group, 1, 3)], 7: [L(q_group, 0, 1)],
                   9: [L(q_group, 1, 1)]}
        lumps01[2] = [L(epilogue, 0, 0, eav00, False, [2, 3])]
        eav01 = attn_pass(0, 1, dmfill, lumps=lumps01,
                          post=[L(epilogue, 0, 0, eav00, False, [0, 1])])
        dmfin0()

        def fix0():
            epilogue(0, 1, eav01, with_dm=True, qs_list=[0, 1])

        def fix0b():
            epilogue(0, 1, eav01, with_dm=True, qs_list=[2, 3])
            for qs in range(4):
                nc.vector.tensor_add(outacc[:, qs, 0:128], outacc[:, qs, 0:128],
                                     dmacc[:, qs, 0:128])

        lump_sched = {
            (1, 1): [(2, L(q_group, 0, 2)), (4, L(q_group, 1, 2))],
            (2, 1): [(2, L(q_group, 0, 3)), (4, L(q_group, 1, 3))],
        }
        prev_post = [fix0]
        ep_half2 = fix0b
        tr0_lump = L(transposes, 0, 0)
        tr1_lump = L(transposes, 0, 1)
        for nqi in range(1, NQ):
            dmfill, dmfin = make_dm_fill(nqi)
            h0_lumps = {}
            if ep_half2:
                h0_lumps[2] = [ep_half2]
            if tr0_lump:
                h0_lumps[4] = [tr0_lump]
            if tr1_lump:
                h0_lumps[7] = [tr1_lump]
            eav_h0 = attn_pass(nqi, 0, dmfill, post=prev_post,
                               lumps=h0_lumps or None)
            tr0_lump = None
            dmfin()

            def pfill(mt, _p=nqi - 1, _l=dict(lump_sched.get((nqi, 1), []))):
                if mt in _l:
                    _l[mt]()
                if 6 <= mt <= 13:
                    proj_group(_p, mt - 6)

            eav_h1 = attn_pass(nqi, 1, pfill,
                               lumps={2: [L(epilogue, nqi, 0, eav_h0, True, [2, 3])],
                                      5: [L(transposes, nqi, 0)]},
                               post=[L(epilogue, nqi, 0, eav_h0, True, [0, 1])])
            prev_post = [L(epilogue, nqi, 1, eav_h1, True, [0, 1])]
            ep_half2 = L(epilogue, nqi, 1, eav_h1, True, [2, 3])
            tr1_lump = L(transposes, nqi, 1) if nqi < NQ - 1 else None
        # ---- tail: last pass's leftovers, pipelined per q-subtile.  W_proj
        # accumulates 128-col partials as each q-subtile's epilogue+transpose
        # lands; ScalarE (idle after the last exp) takes the transpose and
        # half the staging copies.
        _prev = dict(carry)
        carry.clear()
        eav = _prev["eav"]
        tail_pend = list(_prev["pend"])
        nqi = NQ - 1
        tailb = {}

        def eav_qs(mt, et, qs):
            bank = eav[qs // 2]
            base = (qs % 2) * 256
            for h2 in range(2):
                nc.tensor.matmul(
                    bank[:, base + h2 * 65: base + h2 * 65 + 65],
                    lhsT=et[:, h2 * 512 + qs * 128: h2 * 512 + (qs + 1) * 128],
                    rhs=vaug[:, mt, 2 * 1 + h2, :],
                    start=False,
                    stop=(mt == MT - 1 and qs % 2 == 1 and h2 == 1),
                    skip_group_check=True,
                )

        def tpart(co, qs, first, last):
            qti = nqi * 4 + qs
            for jo in range(2):
                nc.tensor.matmul(
                    tailb[co][:, qs * 128:(qs + 1) * 128],
                    lhsT=wp_s[:, jo, co * 128:(co + 1) * 128],
                    rhs=outT[:, jo, qti * 128:(qti + 1) * 128],
                    start=(first and jo == 0), stop=(last and jo == 1),
                    skip_group_check=True,
                )

        def tflush(cos):
            for co in cos:
                so = outp.tile([128, 512], f16, name="so")
                if co % 2 == 0:
                    nc.vector.tensor_copy(so[:, :], tailb[co][:, :])
                else:
                    nc.scalar.copy(so[:, :], tailb[co][:, :])
                nc.sync.dma_start(
                    out=pout[co * 128:(co + 1) * 128, nqi * 512:(nqi + 1) * 512],
                    in_=so[:, :])

        for qs in range(4):
            for mt, et in tail_pend:
                eav_qs(mt, et, qs)
            epilogue(nqi, 1, eav, with_dm=True, qs_list=[qs])
            qti = nqi * 4 + qs
            tr = psS.tile([128, 128], f16, name="tr", tag="psS",
                          padded_shape=[128, 512])
            nc.tensor.transpose(tr[:, :], outacc[:, qti, 128:256], ident_s[:, :])
            nc.scalar.copy(outT[:, 1, qti * 128:(qti + 1) * 128], tr[:, :])
            if qs == 0:
                for co, tg in ((0, "x0"), (1, "x1")):
                    pool = psX
                    tailb[co] = pool.tile([128, 512], f32, name="tb", tag=tg)
            for co in (0, 1):
                tpart(co, qs, first=(qs == 0), last=(qs == 3))
            if qs == 2:
                # a-banks free once ep(qs1) has read them
                for co, tg in ((2, "a0"), (3, "a1")):
                    tailb[co] = psA.tile([128, 512], f32, name="tb", tag=tg)
                for co in (2, 3):
                    for q2 in (0, 1, 2):
                        tpart(co, q2, first=(q2 == 0), last=False)
            elif qs == 3:
                for co in (2, 3):
                    tpart(co, qs, first=False, last=True)
        tflush((0, 1, 2, 3))
        for co, tg in ((4, "x0"), (5, "x1"), (6, "a0"), (7, "a1")):
            pool = psA if tg.startswith("a") else psX
            tailb[co] = pool.tile([128, 512], f32, name="tb", tag=tg)
        for co in (4, 5, 6, 7):
            for qs in range(4):
                tpart(co, qs, first=(qs == 0), last=(qs == 3))
        tflush((4, 5, 6, 7))
    nc.compile()
    return nc


_PROGRAM = None


def _get_program():
    global _PROGRAM
    if _PROGRAM is None:
        _PROGRAM = _build_program()
    return _PROGRAM


def _pack_rows(w, kt):
    # [kt*128, F] -> [128, kt*F]: partition p holds rows p, 128+p, ...
    F = w.shape[1]
    return np.ascontiguousarray(
        w.reshape(kt, 128, F).transpose(1, 0, 2).reshape(128, kt * F))


def _make_in_maps(x, distance_matrix, W_qkv, W_proj):
    ident = np.eye(128, dtype=np.float16)
    in_maps = []
    for core in range(NCORES):
        b, hg = divmod(core, HG)
        sl = slice(hg * DG, (hg + 1) * DG)
        in_maps.append({
            "xT": np.ascontiguousarray(x[b].T).astype(np.float16),
            "wq": _pack_rows(W_qkv[:, sl].astype(np.float16), KT),
            "wk": _pack_rows(W_qkv[:, C + hg * DG:C + (hg + 1) * DG].astype(np.float16), KT),
            "wv": _pack_rows(W_qkv[:, 2 * C + hg * DG:2 * C + (hg + 1) * DG].astype(np.float16), KT),
            "wp": _pack_rows(W_proj[sl, :].astype(np.float16), 2),
            "dmt": _pack_rows((0.5 * distance_matrix[b, 0].T).astype(np.float16), MT),
            "ident": ident,
        })
    return in_maps


def kernel(x, distance_matrix, W_qkv, W_proj, b_proj, _results_hook=None):
    from concourse.bass_utils import run_bass_kernel_spmd

    x = np.asarray(x)
    distance_matrix = np.asarray(distance_matrix)
    W_qkv = np.asarray(W_qkv)
    W_proj = np.asarray(W_proj)
    b_proj = np.asarray(b_proj)
    nc = _get_program()
    in_maps = _make_in_maps(x, distance_matrix, W_qkv, W_proj)
    res = run_bass_kernel_spmd(nc, in_maps, list(range(NCORES)))
    if _results_hook is not None:
        _results_hook(res)
    out = np.zeros((B, N, C), dtype=np.float32)
    for core in range(NCORES):
        b = core // HG
        out[b] += res.results[core]["pout"].T
    out += b_proj[None, None, :].astype(np.float32)
    return out


# revision 46
# speedup vs baseline: 1.0008x; 1.0008x over previous
"""Distributed attention kernel for Trainium2 (8 NeuronCores).

Reference computation (B=2, N=2048, C=1024, H=16, D=64, ALPHA=0.5):
    qkv = x @ W_qkv -> q,k,v [B,H,N,D]
    attn = softmax(q @ k^T / sqrt(D))
    attn = 0.5*dm + 0.5*attn
    out  = (attn @ v).reshape(B,N,C) @ W_proj + b_proj

Sharding: 8 cores = 2 batches x 4 head-groups (4 heads each).
Each core computes its head-group's slice end-to-end, including a partial
projection (row-slice of W_proj); host sums the 4 partials per batch.

On-device layout strategy (per core):
  - x arrives transposed [C, N]; q,k are produced transposed [Dg=256, N]
    (head-dim on partitions); scores are computed transposed
    S^T[m, q] = k^T.T @ q^T so exp runs on ScalarE straight out of PSUM.
  - attn@v runs in the *natural* orientation out[q, d] with the exp tile as
    the stationary operand (lhsT = e^T[m, q-tile 128], rhs = v[m, 65]):
    contraction is the full 128 m-rows AND the output uses all 128 q
    partitions (the transposed form only fills 65 of 128 output rows).
  - v carries an appended column holding 2.0, so out[q, 64] = 2*r_q (the
    softmax denominator); normalization is a per-partition multiply by
    0.5/r_q (vector.reciprocal of the 2r column) fused with the dm@v add
    via scalar_tensor_tensor.
  - dm@v accumulates in the same [q, dg] orientation (lhsT = dm^T tile).
  - The [q, dg] result is transposed back to [dg, q] for the W_proj
    contraction with cheap PE transposes ([128,128] identity matmuls).
  - Schedule: the exp stream on ScalarE (~1.04us per [128,1024] tile, 128
    tiles = 134us) is the secondary critical path after the PE (~168us
    busy), so the first score matmul must issue early and neither engine
    may stall at pass boundaries.  The prologue runs 12 projection groups
    ct-outer while the x tiles stream in (k-jo0 all + q-jo0-nq0 in psS
    slot halves, v m-tiles 0..7 paired in the a/x banks); the remaining
    v/q/k groups, dm@v, and the W_proj groups of the previous q-chunk are
    woven into the per-mt loops of the eight attention passes.  The last
    two e@v emissions and the epilogue of each pass slide into the next
    pass's first iterations ("carry") so the next score stream issues
    immediately.  dm@v bank grabs start at mt>=2 so they never
    head-of-line block scores behind a pending DVE copy.
  - PSUM budget (8 banks): scores [128,1024] x2 bufs = 4; e@v accumulators
    = 2 banks, each holding two q-subtile groups [128,130] at 256-col
    offsets -- only the first matmul per bank uses start=True (hardware
    zeroes the whole 2KB bank region on start), every other group
    accumulates with start=False onto pending-zero bytes; 2 "x" banks
    rotate between prologue groups, dm@v accumulation and W_proj groups;
    transposes ride the psS slots.
  - max-subtraction is skipped: scores are ~N(0,1), exp never overflows.
  - all matmul operands are fp16; PSUM accumulation stays fp32.
"""

import numpy as np

B, N, C, H, D = 2, 2048, 1024, 16, 64
NCORES = 8
HG = 4                # head-groups per batch
HPC = H // HG         # heads per core = 4
DG = HPC * D          # 256: head-group width
SCALE = D ** -0.5

KT = C // 128         # 8 contraction tiles for qkv/x
MT = N // 128         # 16 m (key) tiles
NQ = N // 512         # 4 q-chunks
QT = N // 128         # 16 q-tiles


def _build_program():
    import concourse.bass as bass
    import concourse.bacc as bacc
    import concourse.tile as tile
    from concourse import mybir
    from contextlib import ExitStack

    f32 = mybir.dt.float32
    f16 = mybir.dt.float16
    Exp = mybir.ActivationFunctionType.Exp
    Mult = mybir.AluOpType.mult
    Add = mybir.AluOpType.add

    nc = bacc.Bacc()
    xT = nc.declare_dram_parameter("xT", [C, N], f16, isOutput=False)
    wq = nc.declare_dram_parameter("wq", [128, KT * DG], f16, isOutput=False)
    wk = nc.declare_dram_parameter("wk", [128, KT * DG], f16, isOutput=False)
    wv = nc.declare_dram_parameter("wv", [128, KT * DG], f16, isOutput=False)
    wp = nc.declare_dram_parameter("wp", [128, 2 * C], f16, isOutput=False)
    dmt = nc.declare_dram_parameter("dmt", [128, MT * N], f16, isOutput=False)
    ident = nc.declare_dram_parameter("ident", [128, 128], f16, isOutput=False)
    pout = nc.declare_dram_parameter("pout", [C, N], f16, isOutput=True)

    with tile.TileContext(nc) as tc, ExitStack() as ctx:
        big = ctx.enter_context(tc.tile_pool(name="big", bufs=1))
        epool = ctx.enter_context(tc.tile_pool(name="epool", bufs=8))
        small = ctx.enter_context(tc.tile_pool(name="small", bufs=2))
        outp = ctx.enter_context(tc.tile_pool(name="outp", bufs=4))
        # PSUM: psS 2x[128,1024] = 4 banks, psA 2 banks, psX 2 banks.
        psS = ctx.enter_context(tc.tile_pool(name="psS", bufs=2, space="PSUM"))
        psA = ctx.enter_context(tc.tile_pool(name="psA", bufs=1, space="PSUM"))
        psX = ctx.enter_context(tc.tile_pool(name="psX", bufs=1, space="PSUM"))

        xt = big.tile([128, KT, N], f16)
        wq_s = big.tile([128, KT, DG], f16)
        wk_s = big.tile([128, KT, DG], f16)
        wv_s = big.tile([128, KT, DG], f16)
        wp_s = big.tile([128, 2, C], f16)
        dms = big.tile([128, MT, N], f16)
        qt = big.tile([128, 2, N], f16)
        kt = big.tile([128, 2, N], f16)
        vaug = big.tile([128, MT, HPC, D + 1], f16)
        outacc = big.tile([128, QT, DG], f16)
        dmacc = big.tile([128, QT, DG], f16)
        outT = big.tile([128, 2, N], f16)
        ident_s = big.tile([128, 128], f16)
        ones_sb = big.tile([128, MT * HPC], f32)

        nc.vector.memset(ones_sb[:, :], 2.0)
        nc.vector.tensor_copy(vaug[:, :, :, D], ones_sb[:, :])

        # ---- input DMA: x per-ct (streams the ct-outer prologue), weights
        # one DMA each (host-packed rows), dm in 4 chunks.
        nc.sync.dma_start(out=xt[:, 0, :], in_=xT[0:128, :])
        nc.sync.dma_start(out=wk_s[:, :, :], in_=wk[:, :])
        nc.sync.dma_start(out=wq_s[:, :, :], in_=wq[:, :])
        nc.sync.dma_start(out=wv_s[:, :, :], in_=wv[:, :])
        for ct in range(1, KT):
            nc.sync.dma_start(out=xt[:, ct, :], in_=xT[ct * 128:(ct + 1) * 128, :])
        nc.sync.dma_start(out=ident_s[:, :], in_=ident[:, :])
        nc.sync.dma_start(out=wp_s[:, :, :], in_=wp[:, :])
        for h in range(4):
            nc.sync.dma_start(out=dms[:, 4 * h:4 * h + 4, :],
                              in_=dmt[:, 4 * h * N:(4 * h + 4) * N])

        # ---- prologue: 12 projection groups accumulate ct-outer while the
        # x tiles stream in.  psS slots hold two bank-groups each; the a/x
        # banks hold two v-groups each (single-start-per-bank).
        slotA = psS.tile([128, 1024], f32, name="slotA", tag="psS")
        slotB = psS.tile([128, 1024], f32, name="slotB", tag="psS")
        vslots = {}
        for i, tg in enumerate(("a0", "a1", "x0", "x1")):
            pool = psA if tg.startswith("a") else psX
            vslots[tg] = pool.tile([128, 512], f32, name=f"vs{i}", tag=tg)

        def pro_w(ct, w_s, jo, nqi, dst, first, last):
            nc.tensor.matmul(
                dst, lhsT=w_s[:, ct, jo * 128:(jo + 1) * 128],
                rhs=xt[:, ct, nqi * 512:(nqi + 1) * 512],
                start=first, stop=last, skip_group_check=True)

        def pro_v(ct, mt, first, last):
            bank = vslots[("a0", "a1", "x0", "x1")[mt // 2]]
            nc.tensor.matmul(
                bank[:, (mt % 2) * 256:(mt % 2) * 256 + DG],
                lhsT=xt[:, ct, mt * 128:(mt + 1) * 128],
                rhs=wv_s[:, ct, :],
                start=first and mt % 2 == 0, stop=last,
                skip_group_check=True)

        for ct in range(KT):
            fi, la = ct == 0, ct == KT - 1
            pro_w(ct, wk_s, 0, 0, slotA[:, 0:512], fi, la)
            pro_w(ct, wq_s, 0, 0, slotA[:, 512:1024], fi, la)
            pro_w(ct, wk_s, 0, 1, slotB[:, 0:512], fi, la)
            pro_w(ct, wk_s, 0, 2, slotB[:, 512:1024], fi, la)
            for mt in range(8):
                pro_v(ct, mt, fi, la)
        nc.vector.tensor_copy(kt[:, 0, 0:512], slotA[:, 0:512])
        nc.vector.tensor_scalar_mul(qt[:, 0, 0:512], slotA[:, 512:1024], SCALE)
        nc.vector.tensor_copy(kt[:, 0, 512:1024], slotB[:, 0:512])
        nc.vector.tensor_copy(kt[:, 0, 1024:1536], slotB[:, 512:1024])
        for i, tg in enumerate(("a0", "a1", "x0", "x1")):
            nc.vector.tensor_copy(vaug[:, 2 * i:2 * i + 2, :, 0:D], vslots[tg][:, :])

        # ---- deferred one-time groups, woven into the passes as lumps ----
        def _xtile(tag):
            pool = psA if tag.startswith("a") else (psS if tag == "psS" else psX)
            return pool.tile([128, 512], f32, name=f"lump_{tag}", tag=tag)

        xrot = [0]

        def xtag():
            xrot[0] ^= 1
            return f"x{xrot[0]}"

        def k_group(jo, nqi, tag=None):
            ps = _xtile(tag or xtag())
            for i in range(KT):
                ct = (nqi + i) % KT
                nc.tensor.matmul(
                    ps[:, :],
                    lhsT=wk_s[:, ct, jo * 128:(jo + 1) * 128],
                    rhs=xt[:, ct, nqi * 512:(nqi + 1) * 512],
                    start=(i == 0), stop=(i == KT - 1),
                )
            nc.vector.tensor_copy(kt[:, jo, nqi * 512:(nqi + 1) * 512], ps[:, :])

        def q_group(jo, nqi, tag=None):
            ps = _xtile(tag or xtag())
            for i in range(KT):
                ct = (nqi + i) % KT
                nc.tensor.matmul(
                    ps[:, :],
                    lhsT=wq_s[:, ct, jo * 128:(jo + 1) * 128],
                    rhs=xt[:, ct, nqi * 512:(nqi + 1) * 512],
                    start=(i == 0), stop=(i == KT - 1),
                )
            nc.vector.tensor_scalar_mul(qt[:, jo, nqi * 512:(nqi + 1) * 512], ps[:, :], SCALE)

        def v_group(mt):
            ps = psX.tile([128, DG], f32, name="vps", tag=xtag(),
                          padded_shape=[128, 512])
            for i in range(KT):
                ct = (mt + i) % KT
                nc.tensor.matmul(
                    ps[:, :],
                    lhsT=xt[:, ct, mt * 128:(mt + 1) * 128],
                    rhs=wv_s[:, ct, :],
                    start=(i == 0), stop=(i == KT - 1),
                )
            nc.vector.tensor_copy(vaug[:, mt, :, 0:D], ps[:, :])

        def make_dm_fill(nqi, compact=False):
            state = {}

            def step(mm):
                if not state:
                    state["t"] = [psX.tile([128, 512], f32, name=f"dmps{i}", tag=f"x{i}")
                                  for i in range(2)]
                for qs in range(4):
                    qti = nqi * 4 + qs
                    bank = state["t"][qs // 2]
                    base = (qs % 2) * 256
                    nc.tensor.matmul(
                        bank[:, base:base + DG],
                        lhsT=dms[:, mm, qti * 128:(qti + 1) * 128],
                        rhs=vaug[:, mm, :, 0:D],
                        start=(mm == 0 and qs % 2 == 0),
                        stop=(mm == MT - 1 and qs % 2 == 1),
                        skip_group_check=True,
                    )

            def fill(mt):
                if compact:
                    # 16 steps over mt 10..15 (the x banks host one-time k/q
                    # groups earlier in this pass)
                    sched = {10: (0, 3), 11: (3, 6), 12: (6, 9),
                             13: (9, 12), 14: (12, 14), 15: (14, 16)}
                    if mt in sched:
                        for s in range(*sched[mt]):
                            step(s)
                else:
                    # start at mt 2 so the bank grab never head-of-line
                    # blocks the first score matmuls of the pass
                    if 2 <= mt <= 13:
                        step(mt - 2)
                    elif mt == 14:
                        step(12), step(13)
                    elif mt == 15:
                        step(14), step(15)

            def finish():
                for i in range(2):
                    q0 = nqi * 4 + 2 * i
                    nc.vector.tensor_copy(dmacc[:, q0:q0 + 2, :], state["t"][i][:, :])

            return fill, finish

        def proj_group(nqi, co, tags=("x0", "x1"), act_copy=False):
            qsl = slice(nqi * 512, (nqi + 1) * 512)
            tg = tags[co % len(tags)]
            pool = psA if tg.startswith("a") else psX
            ps = pool.tile([128, 512], f32, name="pps", tag=tg)
            for jo in range(2):
                nc.tensor.matmul(
                    ps[:, :],
                    lhsT=wp_s[:, jo, co * 128:(co + 1) * 128],
                    rhs=outT[:, jo, qsl],
                    start=(jo == 0), stop=(jo == 1),
                )
            so = outp.tile([128, 512], f16, name="so")
            if act_copy:
                nc.scalar.copy(so[:, :], ps[:, :])
            else:
                nc.vector.tensor_copy(so[:, :], ps[:, :])
            nc.sync.dma_start(out=pout[co * 128:(co + 1) * 128, qsl], in_=so[:, :])

        def transposes(nqi, jo):
            # via psS slots (the x banks hold persistent dm accumulators)
            for qs in range(4):
                qti = nqi * 4 + qs
                tr = psS.tile([128, 128], f16, name="tr", tag="psS",
                              padded_shape=[128, 512])
                nc.tensor.transpose(tr[:, :], outacc[:, qti, jo * 128:(jo + 1) * 128],
                                    ident_s[:, :])
                nc.vector.tensor_copy(outT[:, jo, qti * 128:(qti + 1) * 128], tr[:, :])

        # ---- attention pass: scores + exp + e@v for one head pair / q-chunk
        def emit_eav(nqi, hp, eav, mt, et):
            for qs in range(4):
                bank = eav[qs // 2]
                base = (qs % 2) * 256
                for h2 in range(2):
                    nc.tensor.matmul(
                        bank[:, base + h2 * 65: base + h2 * 65 + 65],
                        lhsT=et[:, h2 * 512 + qs * 128: h2 * 512 + (qs + 1) * 128],
                        rhs=vaug[:, mt, 2 * hp + h2, :],
                        start=(mt == 0 and qs % 2 == 0 and h2 == 0),
                        stop=(mt == MT - 1 and qs % 2 == 1 and h2 == 1),
                        skip_group_check=True,
                    )

        # carry: the previous pass's last two e@v emissions and its epilogue
        # slide into the next pass's first iterations, so the next score
        # stream issues immediately and ScalarE never idles at a boundary.
        carry = {}

        def attn_pass(nqi, hp, fill=None, lumps=None, post=(), defer=3):
            qsl = slice(nqi * 512, (nqi + 1) * 512)
            eav = [psA.tile([128, 512], f32, name=f"eav{i}", tag=f"a{i}")
                   for i in range(2)] if not carry else None
            pend = []
            prev = dict(carry) if carry else None
            carry.clear()
            post = list(post)
            for mt in range(MT):
                if lumps and mt in lumps:
                    for th in lumps[mt]:
                        th()
                if fill is not None:
                    fill(mt)
                msl = slice(mt * 128, (mt + 1) * 128)
                sps = psS.tile([128, 1024], f32, name="sps", tag="psS")
                nc.tensor.matmul(sps[:, 0:512], lhsT=kt[0:D, hp, msl],
                                 rhs=qt[0:D, hp, qsl], start=True, stop=True)
                nc.tensor.matmul(sps[:, 512:1024], lhsT=kt[D:128, hp, msl],
                                 rhs=qt[D:128, hp, qsl], start=True, stop=True)
                et = epool.tile([128, 1024], f16, name="et", tag="et")
                nc.scalar.activation(et[:, :], sps[:, :], Exp)
                pend.append((mt, et))
                if prev is not None:
                    if prev["pend"]:
                        emit_eav(prev["nqi"], prev["hp"], prev["eav"],
                                 *prev["pend"].pop(0))
                    if not prev["pend"]:
                        for th in post:
                            th()
                        post = []
                        prev = None
                        eav = [psA.tile([128, 512], f32, name=f"eav{i}", tag=f"a{i}")
                               for i in range(2)]
                elif len(pend) > defer:
                    emit_eav(nqi, hp, eav, *pend.pop(0))
            while len(pend) > 2:
                emit_eav(nqi, hp, eav, *pend.pop(0))
            carry.update(dict(nqi=nqi, hp=hp, eav=eav, pend=pend))
            return eav

        def flush_carry():
            prev = dict(carry)
            carry.clear()
            while prev["pend"]:
                emit_eav(prev["nqi"], prev["hp"], prev["eav"], *prev["pend"].pop(0))
            return prev["eav"]

        def epilogue(nqi, hp, eav, with_dm, qs_list=range(4)):
            for qs in qs_list:
                qti = nqi * 4 + qs
                bank = eav[qs // 2]
                base = (qs % 2) * 256
                rec = small.tile([128, 2], f32, name="rec", tag="rec")
                with nc.allow_low_precision(reason="0.5/r per-q reciprocal"):
                    for h2 in range(2):
                        nc.vector.reciprocal(rec[:, h2:h2 + 1],
                                             bank[:, base + h2 * 65 + 64: base + h2 * 65 + 65])
                for h2 in range(2):
                    col = base + h2 * 65
                    dst = outacc[:, qti, (2 * hp + h2) * 64:(2 * hp + h2 + 1) * 64]
                    if with_dm:
                        nc.vector.scalar_tensor_tensor(
                            dst, bank[:, col:col + 64], rec[:, h2:h2 + 1],
                            dmacc[:, qti, (2 * hp + h2) * 64:(2 * hp + h2 + 1) * 64],
                            op0=Mult, op1=Add)
                    else:
                        nc.vector.tensor_scalar_mul(dst, bank[:, col:col + 64],
                                                    rec[:, h2:h2 + 1])

        # ---- main schedule ----
        L = lambda f, *a, **k: (lambda: f(*a, **k))
        lumps00 = {
            3: [L(v_group, 8)], 4: [L(v_group, 9)],
            5: [L(k_group, 0, 3)], 6: [L(k_group, 1, 0)],
            7: [L(v_group, 10)], 8: [L(q_group, 1, 0)],
            9: [L(v_group, 11)], 10: [L(v_group, 12)],
            11: [L(v_group, 13)], 12: [L(v_group, 14)],
            13: [L(v_group, 15)],
        }
        eav00 = attn_pass(0, 0, lumps=lumps00)
        dmfill, dmfin0 = make_dm_fill(0, compact=True)
        lumps01 = {1: [L(k_group, 1, 1)], 3: [L(k_group, 1, 2)],
                   5: [L(k_group, 1, 3)], 7: [L(q_group, 0, 1)],
                   9: [L(q_group, 1, 1)]}
        lumps01[2] = [L(epilogue, 0, 0, eav00, False, [2, 3])]
        eav01 = attn_pass(0, 1, dmfill, lumps=lumps01,
                          post=[L(epilogue, 0, 0, eav00, False, [0, 1])])
        dmfin0()

        def fix0():
            epilogue(0, 1, eav01, with_dm=True, qs_list=[0, 1])

        def fix0b():
            epilogue(0, 1, eav01, with_dm=True, qs_list=[2, 3])
            for qs in range(4):
                nc.vector.tensor_add(outacc[:, qs, 0:128], outacc[:, qs, 0:128],
                                     dmacc[:, qs, 0:128])

        lump_sched = {
            (1, 1): [(2, L(q_group, 0, 2)), (4, L(q_group, 1, 2))],
            (2, 1): [(2, L(q_group, 0, 3)), (4, L(q_group, 1, 3))],
        }
        prev_post = [fix0]
        ep_half2 = fix0b
        tr0_lump = L(transposes, 0, 0)
        tr1_lump = L(transposes, 0, 1)
        for nqi in range(1, NQ):
            dmfill, dmfin = make_dm_fill(nqi)
            h0_lumps = {}
            if ep_half2:
                h0_lumps[2] = [ep_half2]
            if tr0_lump:
                h0_lumps[4] = [tr0_lump]
            if tr1_lump:
                h0_lumps[7] = [tr1_lump]
            eav_h0 = attn_pass(nqi, 0, dmfill, post=prev_post,
                               lumps=h0_lumps or None)
            tr0_lump = None
            dmfin()

            def pfill(mt, _p=nqi - 1, _l=dict(lump_sched.get((nqi, 1), []))):
                if mt in _l:
                    _l[mt]()
                if 6 <= mt <= 13:
                    proj_group(_p, mt - 6)

            eav_h1 = attn_pass(nqi, 1, pfill,
                               lumps={2: [L(epilogue, nqi, 0, eav_h0, True, [2, 3])],
                                      5: [L(transposes, nqi, 0)]},
                               post=[L(epilogue, nqi, 0, eav_h0, True, [0, 1])])
            prev_post = [L(epilogue, nqi, 1, eav_h1, True, [0, 1])]
            ep_half2 = L(epilogue, nqi, 1, eav_h1, True, [2, 3])
            tr1_lump = L(transposes, nqi, 1) if nqi < NQ - 1 else None
        # ---- tail: last pass's leftovers, pipelined per q-subtile.  W_proj
        # accumulates 128-col partials as each q-subtile's epilogue+transpose
        # lands; ScalarE (idle after the last exp) takes the transpose and
        # half the staging copies.
        eav = flush_carry()
        nqi = NQ - 1
        tailb = {}

        def tpart(co, qs, first, last):
            qti = nqi * 4 + qs
            for jo in range(2):
                nc.tensor.matmul(
                    tailb[co][:, qs * 128:(qs + 1) * 128],
                    lhsT=wp_s[:, jo, co * 128:(co + 1) * 128],
                    rhs=outT[:, jo, qti * 128:(qti + 1) * 128],
                    start=(first and jo == 0), stop=(last and jo == 1),
                    skip_group_check=True,
                )

        def tflush(cos):
            for co in cos:
                so = outp.tile([128, 512], f16, name="so")
                if co % 2 == 0:
                    nc.vector.tensor_copy(so[:, :], tailb[co][:, :])
                else:
                    nc.scalar.copy(so[:, :], tailb[co][:, :])
                nc.sync.dma_start(
                    out=pout[co * 128:(co + 1) * 128, nqi * 512:(nqi + 1) * 512],
                    in_=so[:, :])

        for qs in range(4):
            epilogue(nqi, 1, eav, with_dm=True, qs_list=[qs])
            qti = nqi * 4 + qs
            tr = psS.tile([128, 128], f16, name="tr", tag="psS",
                          padded_shape=[128, 512])
            nc.tensor.transpose(tr[:, :], outacc[:, qti, 128:256], ident_s[:, :])
            nc.scalar.copy(outT[:, 1, qti * 128:(qti + 1) * 128], tr[:, :])
            if qs == 0:
                for co, tg in ((0, "x0"), (1, "x1")):
                    pool = psX
                    tailb[co] = pool.tile([128, 512], f32, name="tb", tag=tg)
            for co in (0, 1):
                tpart(co, qs, first=(qs == 0), last=(qs == 3))
            if qs == 2:
                # a-banks free once ep(qs1) has read them
                for co, tg in ((2, "a0"), (3, "a1")):
                    tailb[co] = psA.tile([128, 512], f32, name="tb", tag=tg)
                for co in (2, 3):
                    for q2 in (0, 1, 2):
                        tpart(co, q2, first=(q2 == 0), last=False)
            elif qs == 3:
                for co in (2, 3):
                    tpart(co, qs, first=False, last=True)
        tflush((0, 1, 2, 3))
        for co, tg in ((4, "x0"), (5, "x1"), (6, "a0"), (7, "a1")):
            pool = psA if tg.startswith("a") else psX
            tailb[co] = pool.tile([128, 512], f32, name="tb", tag=tg)
        for co in (4, 5, 6, 7):
            for qs in range(4):
                tpart(co, qs, first=(qs == 0), last=(qs == 3))
        tflush((4, 5, 6, 7))
    nc.compile()
    return nc


_PROGRAM = None


def _get_program():
    global _PROGRAM
    if _PROGRAM is None:
        _PROGRAM = _build_program()
    return _PROGRAM


def _pack_rows(w, kt):
    # [kt*128, F] -> [128, kt*F]: partition p holds rows p, 128+p, ...
    F = w.shape[1]
    return np.ascontiguousarray(
        w.reshape(kt, 128, F).transpose(1, 0, 2).reshape(128, kt * F))


def _make_in_maps(x, distance_matrix, W_qkv, W_proj):
    ident = np.eye(128, dtype=np.float16)
    in_maps = []
    for core in range(NCORES):
        b, hg = divmod(core, HG)
        sl = slice(hg * DG, (hg + 1) * DG)
        in_maps.append({
            "xT": np.ascontiguousarray(x[b].T).astype(np.float16),
            "wq": _pack_rows(W_qkv[:, sl].astype(np.float16), KT),
            "wk": _pack_rows(W_qkv[:, C + hg * DG:C + (hg + 1) * DG].astype(np.float16), KT),
            "wv": _pack_rows(W_qkv[:, 2 * C + hg * DG:2 * C + (hg + 1) * DG].astype(np.float16), KT),
            "wp": _pack_rows(W_proj[sl, :].astype(np.float16), 2),
            "dmt": _pack_rows((0.5 * distance_matrix[b, 0].T).astype(np.float16), MT),
            "ident": ident,
        })
    return in_maps


def kernel(x, distance_matrix, W_qkv, W_proj, b_proj, _results_hook=None):
    from concourse.bass_utils import run_bass_kernel_spmd

    x = np.asarray(x)
    distance_matrix = np.asarray(distance_matrix)
    W_qkv = np.asarray(W_qkv)
    W_proj = np.asarray(W_proj)
    b_proj = np.asarray(b_proj)
    nc = _get_program()
    in_maps = _make_in_maps(x, distance_matrix, W_qkv, W_proj)
    res = run_bass_kernel_spmd(nc, in_maps, list(range(NCORES)))
    if _results_hook is not None:
        _results_hook(res)
    out = np.zeros((B, N, C), dtype=np.float32)
    for core in range(NCORES):
        b = core // HG
        out[b] += res.results[core]["pout"].T
    out += b_proj[None, None, :].astype(np.float32)
    return out


# revision 47
# speedup vs baseline: 1.0050x; 1.0041x over previous
"""Distributed attention kernel for Trainium2 (8 NeuronCores).

Reference computation (B=2, N=2048, C=1024, H=16, D=64, ALPHA=0.5):
    qkv = x @ W_qkv -> q,k,v [B,H,N,D]
    attn = softmax(q @ k^T / sqrt(D))
    attn = 0.5*dm + 0.5*attn
    out  = (attn @ v).reshape(B,N,C) @ W_proj + b_proj

Sharding: 8 cores = 2 batches x 4 head-groups (4 heads each).
Each core computes its head-group's slice end-to-end, including a partial
projection (row-slice of W_proj); host sums the 4 partials per batch.

On-device layout strategy (per core):
  - x arrives transposed [C, N]; q,k are produced transposed [Dg=256, N]
    (head-dim on partitions); scores are computed transposed
    S^T[m, q] = k^T.T @ q^T so exp runs on ScalarE straight out of PSUM.
  - attn@v runs in the *natural* orientation out[q, d] with the exp tile as
    the stationary operand (lhsT = e^T[m, q-tile 128], rhs = v[m, 65]):
    contraction is the full 128 m-rows AND the output uses all 128 q
    partitions (the transposed form only fills 65 of 128 output rows).
  - v carries an appended column holding 2.0, so out[q, 64] = 2*r_q (the
    softmax denominator); normalization is a per-partition multiply by
    0.5/r_q (vector.reciprocal of the 2r column) fused with the dm@v add
    via scalar_tensor_tensor.
  - dm@v accumulates in the same [q, dg] orientation (lhsT = dm^T tile).
  - The [q, dg] result is transposed back to [dg, q] for the W_proj
    contraction with cheap PE transposes ([128,128] identity matmuls).
  - Schedule: the exp stream on ScalarE (~1.04us per [128,1024] tile, 128
    tiles = 134us) is the secondary critical path after the PE (~168us
    busy), so the first score matmul must issue early and neither engine
    may stall at pass boundaries.  The prologue runs 12 projection groups
    ct-outer while the x tiles stream in (k-jo0 all + q-jo0-nq0 in psS
    slot halves, v m-tiles 0..7 paired in the a/x banks); the remaining
    v/q/k groups, dm@v, and the W_proj groups of the previous q-chunk are
    woven into the per-mt loops of the eight attention passes.  The last
    two e@v emissions and the epilogue of each pass slide into the next
    pass's first iterations ("carry") so the next score stream issues
    immediately.  dm@v bank grabs start at mt>=2 so they never
    head-of-line block scores behind a pending DVE copy.
  - PSUM budget (8 banks): scores [128,1024] x2 bufs = 4; e@v accumulators
    = 2 banks, each holding two q-subtile groups [128,130] at 256-col
    offsets -- only the first matmul per bank uses start=True (hardware
    zeroes the whole 2KB bank region on start), every other group
    accumulates with start=False onto pending-zero bytes; 2 "x" banks
    rotate between prologue groups, dm@v accumulation and W_proj groups;
    transposes ride the psS slots.
  - max-subtraction is skipped: scores are ~N(0,1), exp never overflows.
  - all matmul operands are fp16; PSUM accumulation stays fp32.
"""

import numpy as np

B, N, C, H, D = 2, 2048, 1024, 16, 64
NCORES = 8
HG = 4                # head-groups per batch
HPC = H // HG         # heads per core = 4
DG = HPC * D          # 256: head-group width
SCALE = D ** -0.5

KT = C // 128         # 8 contraction tiles for qkv/x
MT = N // 128         # 16 m (key) tiles
NQ = N // 512         # 4 q-chunks
QT = N // 128         # 16 q-tiles


def _build_program():
    import concourse.bass as bass
    import concourse.bacc as bacc
    import concourse.tile as tile
    from concourse import mybir
    from contextlib import ExitStack

    f32 = mybir.dt.float32
    f16 = mybir.dt.float16
    Exp = mybir.ActivationFunctionType.Exp
    Mult = mybir.AluOpType.mult
    Add = mybir.AluOpType.add

    nc = bacc.Bacc()
    xT = nc.declare_dram_parameter("xT", [C, N], f16, isOutput=False)
    wq = nc.declare_dram_parameter("wq", [128, KT * DG], f16, isOutput=False)
    wk = nc.declare_dram_parameter("wk", [128, KT * DG], f16, isOutput=False)
    wv = nc.declare_dram_parameter("wv", [128, KT * DG], f16, isOutput=False)
    wp = nc.declare_dram_parameter("wp", [128, 2 * C], f16, isOutput=False)
    dmt = nc.declare_dram_parameter("dmt", [128, MT * N], f16, isOutput=False)
    ident = nc.declare_dram_parameter("ident", [128, 128], f16, isOutput=False)
    pout = nc.declare_dram_parameter("pout", [C, N], f16, isOutput=True)

    with tile.TileContext(nc) as tc, ExitStack() as ctx:
        big = ctx.enter_context(tc.tile_pool(name="big", bufs=1))
        epool = ctx.enter_context(tc.tile_pool(name="epool", bufs=8))
        small = ctx.enter_context(tc.tile_pool(name="small", bufs=2))
        outp = ctx.enter_context(tc.tile_pool(name="outp", bufs=4))
        # PSUM: psS 2x[128,1024] = 4 banks, psA 2 banks, psX 2 banks.
        psS = ctx.enter_context(tc.tile_pool(name="psS", bufs=2, space="PSUM"))
        psA = ctx.enter_context(tc.tile_pool(name="psA", bufs=1, space="PSUM"))
        psX = ctx.enter_context(tc.tile_pool(name="psX", bufs=1, space="PSUM"))

        xt = big.tile([128, KT, N], f16)
        wq_s = big.tile([128, 2, KT, 128], f16)
        wk_s = big.tile([128, 2, KT, 128], f16)
        wv_s = big.tile([128, KT, DG], f16)
        wp_s = big.tile([128, 2, C], f16)
        dms = big.tile([128, MT, N], f16)
        qt = big.tile([128, 2, N], f16)
        kt = big.tile([128, 2, N], f16)
        vaug = big.tile([128, MT, HPC, D + 1], f16)
        outacc = big.tile([128, QT, DG], f16)
        dmacc = big.tile([128, QT, DG], f16)
        outT = big.tile([128, 2, N], f16)
        ident_s = big.tile([128, 128], f16)
        ones_sb = big.tile([128, MT * HPC], f32)

        nc.vector.memset(ones_sb[:, :], 2.0)
        nc.vector.tensor_copy(vaug[:, :, :, D], ones_sb[:, :])

        # ---- input DMA: x per-ct (streams the ct-outer prologue), weights
        # one DMA each (host-packed rows), dm in 4 chunks.
        nc.sync.dma_start(out=xt[:, 0, :], in_=xT[0:128, :])
        nc.sync.dma_start(out=wk_s[:, 0, :, :], in_=wk[:, 0:KT * 128])
        nc.sync.dma_start(out=wq_s[:, 0, :, :], in_=wq[:, 0:KT * 128])
        nc.sync.dma_start(out=wv_s[:, :, :], in_=wv[:, :])
        for ct in range(1, KT):
            nc.sync.dma_start(out=xt[:, ct, :], in_=xT[ct * 128:(ct + 1) * 128, :])
        nc.sync.dma_start(out=wk_s[:, 1, :, :], in_=wk[:, KT * 128:])
        nc.sync.dma_start(out=wq_s[:, 1, :, :], in_=wq[:, KT * 128:])
        nc.sync.dma_start(out=ident_s[:, :], in_=ident[:, :])
        nc.sync.dma_start(out=wp_s[:, :, :], in_=wp[:, :])
        for h in range(4):
            nc.sync.dma_start(out=dms[:, 4 * h:4 * h + 4, :],
                              in_=dmt[:, 4 * h * N:(4 * h + 4) * N])

        # ---- prologue: 12 projection groups accumulate ct-outer while the
        # x tiles stream in.  psS slots hold two bank-groups each; the a/x
        # banks hold two v-groups each (single-start-per-bank).
        slotA = psS.tile([128, 1024], f32, name="slotA", tag="psS")
        slotB = psS.tile([128, 1024], f32, name="slotB", tag="psS")
        vslots = {}
        for i, tg in enumerate(("a0", "a1", "x0", "x1")):
            pool = psA if tg.startswith("a") else psX
            vslots[tg] = pool.tile([128, 512], f32, name=f"vs{i}", tag=tg)

        def pro_w(ct, w_s, jo, nqi, dst, first, last):
            nc.tensor.matmul(
                dst, lhsT=w_s[:, jo, ct, :],
                rhs=xt[:, ct, nqi * 512:(nqi + 1) * 512],
                start=first, stop=last, skip_group_check=True)

        def pro_v(ct, mt, first, last):
            bank = vslots[("a0", "a1", "x0", "x1")[mt // 2]]
            nc.tensor.matmul(
                bank[:, (mt % 2) * 256:(mt % 2) * 256 + DG],
                lhsT=xt[:, ct, mt * 128:(mt + 1) * 128],
                rhs=wv_s[:, ct, :],
                start=first and mt % 2 == 0, stop=last,
                skip_group_check=True)

        for ct in range(KT):
            fi, la = ct == 0, ct == KT - 1
            pro_w(ct, wk_s, 0, 0, slotA[:, 0:512], fi, la)
            pro_w(ct, wq_s, 0, 0, slotA[:, 512:1024], fi, la)
            pro_w(ct, wk_s, 0, 1, slotB[:, 0:512], fi, la)
            pro_w(ct, wk_s, 0, 2, slotB[:, 512:1024], fi, la)
            for mt in range(8):
                pro_v(ct, mt, fi, la)
        nc.vector.tensor_copy(kt[:, 0, 0:512], slotA[:, 0:512])
        nc.vector.tensor_scalar_mul(qt[:, 0, 0:512], slotA[:, 512:1024], SCALE)
        nc.vector.tensor_copy(kt[:, 0, 512:1024], slotB[:, 0:512])
        nc.vector.tensor_copy(kt[:, 0, 1024:1536], slotB[:, 512:1024])
        for i, tg in enumerate(("a0", "a1", "x0", "x1")):
            nc.vector.tensor_copy(vaug[:, 2 * i:2 * i + 2, :, 0:D], vslots[tg][:, :])

        # ---- deferred one-time groups, woven into the passes as lumps ----
        def _xtile(tag):
            pool = psA if tag.startswith("a") else (psS if tag == "psS" else psX)
            return pool.tile([128, 512], f32, name=f"lump_{tag}", tag=tag)

        xrot = [0]

        def xtag():
            xrot[0] ^= 1
            return f"x{xrot[0]}"

        def k_group(jo, nqi, tag=None):
            ps = _xtile(tag or xtag())
            for i in range(KT):
                ct = (nqi + i) % KT
                nc.tensor.matmul(
                    ps[:, :],
                    lhsT=wk_s[:, jo, ct, :],
                    rhs=xt[:, ct, nqi * 512:(nqi + 1) * 512],
                    start=(i == 0), stop=(i == KT - 1),
                )
            nc.vector.tensor_copy(kt[:, jo, nqi * 512:(nqi + 1) * 512], ps[:, :])

        def q_group(jo, nqi, tag=None):
            ps = _xtile(tag or xtag())
            for i in range(KT):
                ct = (nqi + i) % KT
                nc.tensor.matmul(
                    ps[:, :],
                    lhsT=wq_s[:, jo, ct, :],
                    rhs=xt[:, ct, nqi * 512:(nqi + 1) * 512],
                    start=(i == 0), stop=(i == KT - 1),
                )
            nc.vector.tensor_scalar_mul(qt[:, jo, nqi * 512:(nqi + 1) * 512], ps[:, :], SCALE)

        def v_group(mt):
            ps = psX.tile([128, DG], f32, name="vps", tag=xtag(),
                          padded_shape=[128, 512])
            for i in range(KT):
                ct = (mt + i) % KT
                nc.tensor.matmul(
                    ps[:, :],
                    lhsT=xt[:, ct, mt * 128:(mt + 1) * 128],
                    rhs=wv_s[:, ct, :],
                    start=(i == 0), stop=(i == KT - 1),
                )
            nc.vector.tensor_copy(vaug[:, mt, :, 0:D], ps[:, :])

        def make_dm_fill(nqi, compact=False):
            state = {}

            def step(mm):
                if not state:
                    state["t"] = [psX.tile([128, 512], f32, name=f"dmps{i}", tag=f"x{i}")
                                  for i in range(2)]
                for qs in range(4):
                    qti = nqi * 4 + qs
                    bank = state["t"][qs // 2]
                    base = (qs % 2) * 256
                    nc.tensor.matmul(
                        bank[:, base:base + DG],
                        lhsT=dms[:, mm, qti * 128:(qti + 1) * 128],
                        rhs=vaug[:, mm, :, 0:D],
                        start=(mm == 0 and qs % 2 == 0),
                        stop=(mm == MT - 1 and qs % 2 == 1),
                        skip_group_check=True,
                    )

            def fill(mt):
                if compact:
                    # 16 steps over mt 10..15 (the x banks host one-time k/q
                    # groups earlier in this pass)
                    sched = {10: (0, 3), 11: (3, 6), 12: (6, 9),
                             13: (9, 12), 14: (12, 14), 15: (14, 16)}
                    if mt in sched:
                        for s in range(*sched[mt]):
                            step(s)
                else:
                    # start at mt 2 so the bank grab never head-of-line
                    # blocks the first score matmuls of the pass
                    if 2 <= mt <= 13:
                        step(mt - 2)
                    elif mt == 14:
                        step(12), step(13)
                    elif mt == 15:
                        step(14), step(15)

            def finish():
                for i in range(2):
                    q0 = nqi * 4 + 2 * i
                    nc.vector.tensor_copy(dmacc[:, q0:q0 + 2, :], state["t"][i][:, :])

            return fill, finish

        def proj_group(nqi, co, tags=("x0", "x1"), act_copy=False):
            qsl = slice(nqi * 512, (nqi + 1) * 512)
            tg = tags[co % len(tags)]
            pool = psA if tg.startswith("a") else psX
            ps = pool.tile([128, 512], f32, name="pps", tag=tg)
            for jo in range(2):
                nc.tensor.matmul(
                    ps[:, :],
                    lhsT=wp_s[:, jo, co * 128:(co + 1) * 128],
                    rhs=outT[:, jo, qsl],
                    start=(jo == 0), stop=(jo == 1),
                )
            so = outp.tile([128, 512], f16, name="so")
            if act_copy:
                nc.scalar.copy(so[:, :], ps[:, :])
            else:
                nc.vector.tensor_copy(so[:, :], ps[:, :])
            nc.sync.dma_start(out=pout[co * 128:(co + 1) * 128, qsl], in_=so[:, :])

        def transposes(nqi, jo):
            # via psS slots (the x banks hold persistent dm accumulators)
            for qs in range(4):
                qti = nqi * 4 + qs
                tr = psS.tile([128, 128], f16, name="tr", tag="psS",
                              padded_shape=[128, 512])
                nc.tensor.transpose(tr[:, :], outacc[:, qti, jo * 128:(jo + 1) * 128],
                                    ident_s[:, :])
                nc.vector.tensor_copy(outT[:, jo, qti * 128:(qti + 1) * 128], tr[:, :])

        # ---- attention pass: scores + exp + e@v for one head pair / q-chunk
        def emit_eav(nqi, hp, eav, mt, et):
            for qs in range(4):
                bank = eav[qs // 2]
                base = (qs % 2) * 256
                for h2 in range(2):
                    nc.tensor.matmul(
                        bank[:, base + h2 * 65: base + h2 * 65 + 65],
                        lhsT=et[:, h2 * 512 + qs * 128: h2 * 512 + (qs + 1) * 128],
                        rhs=vaug[:, mt, 2 * hp + h2, :],
                        start=(mt == 0 and qs % 2 == 0 and h2 == 0),
                        stop=(mt == MT - 1 and qs % 2 == 1 and h2 == 1),
                        skip_group_check=True,
                    )

        # carry: the previous pass's last two e@v emissions and its epilogue
        # slide into the next pass's first iterations, so the next score
        # stream issues immediately and ScalarE never idles at a boundary.
        carry = {}

        def attn_pass(nqi, hp, fill=None, lumps=None, post=(), defer=3):
            qsl = slice(nqi * 512, (nqi + 1) * 512)
            eav = [psA.tile([128, 512], f32, name=f"eav{i}", tag=f"a{i}")
                   for i in range(2)] if not carry else None
            pend = []
            prev = dict(carry) if carry else None
            carry.clear()
            post = list(post)
            for mt in range(MT):
                if lumps and mt in lumps:
                    for th in lumps[mt]:
                        th()
                if fill is not None:
                    fill(mt)
                msl = slice(mt * 128, (mt + 1) * 128)
                sps = psS.tile([128, 1024], f32, name="sps", tag="psS")
                nc.tensor.matmul(sps[:, 0:512], lhsT=kt[0:D, hp, msl],
                                 rhs=qt[0:D, hp, qsl], start=True, stop=True)
                nc.tensor.matmul(sps[:, 512:1024], lhsT=kt[D:128, hp, msl],
                                 rhs=qt[D:128, hp, qsl], start=True, stop=True)
                et = epool.tile([128, 1024], f16, name="et", tag="et")
                nc.scalar.activation(et[:, :], sps[:, :], Exp)
                pend.append((mt, et))
                if prev is not None:
                    if prev["pend"]:
                        emit_eav(prev["nqi"], prev["hp"], prev["eav"],
                                 *prev["pend"].pop(0))
                    if not prev["pend"]:
                        for th in post:
                            th()
                        post = []
                        prev = None
                        eav = [psA.tile([128, 512], f32, name=f"eav{i}", tag=f"a{i}")
                               for i in range(2)]
                elif len(pend) > defer:
                    emit_eav(nqi, hp, eav, *pend.pop(0))
            while len(pend) > 2:
                emit_eav(nqi, hp, eav, *pend.pop(0))
            carry.update(dict(nqi=nqi, hp=hp, eav=eav, pend=pend))
            return eav

        def flush_carry():
            prev = dict(carry)
            carry.clear()
            while prev["pend"]:
                emit_eav(prev["nqi"], prev["hp"], prev["eav"], *prev["pend"].pop(0))
            return prev["eav"]

        def epilogue(nqi, hp, eav, with_dm, qs_list=range(4)):
            for qs in qs_list:
                qti = nqi * 4 + qs
                bank = eav[qs // 2]
                base = (qs % 2) * 256
                rec = small.tile([128, 2], f32, name="rec", tag="rec")
                with nc.allow_low_precision(reason="0.5/r per-q reciprocal"):
                    for h2 in range(2):
                        nc.vector.reciprocal(rec[:, h2:h2 + 1],
                                             bank[:, base + h2 * 65 + 64: base + h2 * 65 + 65])
                for h2 in range(2):
                    col = base + h2 * 65
                    dst = outacc[:, qti, (2 * hp + h2) * 64:(2 * hp + h2 + 1) * 64]
                    if with_dm:
                        nc.vector.scalar_tensor_tensor(
                            dst, bank[:, col:col + 64], rec[:, h2:h2 + 1],
                            dmacc[:, qti, (2 * hp + h2) * 64:(2 * hp + h2 + 1) * 64],
                            op0=Mult, op1=Add)
                    else:
                        nc.vector.tensor_scalar_mul(dst, bank[:, col:col + 64],
                                                    rec[:, h2:h2 + 1])

        # ---- main schedule ----
        L = lambda f, *a, **k: (lambda: f(*a, **k))
        lumps00 = {
            3: [L(v_group, 8)], 4: [L(v_group, 9)],
            5: [L(k_group, 0, 3)], 6: [L(k_group, 1, 0)],
            7: [L(v_group, 10)], 8: [L(q_group, 1, 0)],
            9: [L(v_group, 11)], 10: [L(v_group, 12)],
            11: [L(v_group, 13)], 12: [L(v_group, 14)],
            13: [L(v_group, 15)],
        }
        eav00 = attn_pass(0, 0, lumps=lumps00)
        dmfill, dmfin0 = make_dm_fill(0, compact=True)
        lumps01 = {1: [L(k_group, 1, 1)], 3: [L(k_group, 1, 2)],
                   5: [L(k_group, 1, 3)], 7: [L(q_group, 0, 1)],
                   9: [L(q_group, 1, 1)]}
        lumps01[2] = [L(epilogue, 0, 0, eav00, False, [2, 3])]
        eav01 = attn_pass(0, 1, dmfill, lumps=lumps01,
                          post=[L(epilogue, 0, 0, eav00, False, [0, 1])])
        dmfin0()

        def fix0():
            epilogue(0, 1, eav01, with_dm=True, qs_list=[0, 1])

        def fix0b():
            epilogue(0, 1, eav01, with_dm=True, qs_list=[2, 3])
            for qs in range(4):
                nc.vector.tensor_add(outacc[:, qs, 0:128], outacc[:, qs, 0:128],
                                     dmacc[:, qs, 0:128])

        lump_sched = {
            (1, 1): [(2, L(q_group, 0, 2)), (4, L(q_group, 1, 2))],
            (2, 1): [(2, L(q_group, 0, 3)), (4, L(q_group, 1, 3))],
        }
        prev_post = [fix0]
        ep_half2 = fix0b
        tr0_lump = L(transposes, 0, 0)
        tr1_lump = L(transposes, 0, 1)
        for nqi in range(1, NQ):
            dmfill, dmfin = make_dm_fill(nqi)
            h0_lumps = {}
            if ep_half2:
                h0_lumps[2] = [ep_half2]
            if tr0_lump:
                h0_lumps[4] = [tr0_lump]
            if tr1_lump:
                h0_lumps[7] = [tr1_lump]
            eav_h0 = attn_pass(nqi, 0, dmfill, post=prev_post,
                               lumps=h0_lumps or None)
            tr0_lump = None
            dmfin()

            def pfill(mt, _p=nqi - 1, _l=dict(lump_sched.get((nqi, 1), []))):
                if mt in _l:
                    _l[mt]()
                if 6 <= mt <= 13:
                    proj_group(_p, mt - 6)

            eav_h1 = attn_pass(nqi, 1, pfill,
                               lumps={2: [L(epilogue, nqi, 0, eav_h0, True, [2, 3])],
                                      5: [L(transposes, nqi, 0)]},
                               post=[L(epilogue, nqi, 0, eav_h0, True, [0, 1])])
            prev_post = [L(epilogue, nqi, 1, eav_h1, True, [0, 1])]
            ep_half2 = L(epilogue, nqi, 1, eav_h1, True, [2, 3])
            tr1_lump = L(transposes, nqi, 1) if nqi < NQ - 1 else None
        # ---- tail: last pass's leftovers, pipelined per q-subtile.  W_proj
        # accumulates 128-col partials as each q-subtile's epilogue+transpose
        # lands; ScalarE (idle after the last exp) takes the transpose and
        # half the staging copies.
        eav = flush_carry()
        nqi = NQ - 1
        tailb = {}

        def tpart(co, qs, first, last):
            qti = nqi * 4 + qs
            for jo in range(2):
                nc.tensor.matmul(
                    tailb[co][:, qs * 128:(qs + 1) * 128],
                    lhsT=wp_s[:, jo, co * 128:(co + 1) * 128],
                    rhs=outT[:, jo, qti * 128:(qti + 1) * 128],
                    start=(first and jo == 0), stop=(last and jo == 1),
                    skip_group_check=True,
                )

        def tflush(cos):
            for co in cos:
                so = outp.tile([128, 512], f16, name="so")
                if co % 2 == 0:
                    nc.vector.tensor_copy(so[:, :], tailb[co][:, :])
                else:
                    nc.scalar.copy(so[:, :], tailb[co][:, :])
                nc.sync.dma_start(
                    out=pout[co * 128:(co + 1) * 128, nqi * 512:(nqi + 1) * 512],
                    in_=so[:, :])

        for qs in range(4):
            epilogue(nqi, 1, eav, with_dm=True, qs_list=[qs])
            qti = nqi * 4 + qs
            tr = psS.tile([128, 128], f16, name="tr", tag="psS",
                          padded_shape=[128, 512])
            nc.tensor.transpose(tr[:, :], outacc[:, qti, 128:256], ident_s[:, :])
            nc.scalar.copy(outT[:, 1, qti * 128:(qti + 1) * 128], tr[:, :])
            if qs == 0:
                for co, tg in ((0, "x0"), (1, "x1")):
                    pool = psX
                    tailb[co] = pool.tile([128, 512], f32, name="tb", tag=tg)
            for co in (0, 1):
                tpart(co, qs, first=(qs == 0), last=(qs == 3))
            if qs == 2:
                # a-banks free once ep(qs1) has read them
                for co, tg in ((2, "a0"), (3, "a1")):
                    tailb[co] = psA.tile([128, 512], f32, name="tb", tag=tg)
                for co in (2, 3):
                    for q2 in (0, 1, 2):
                        tpart(co, q2, first=(q2 == 0), last=False)
            elif qs == 3:
                for co in (2, 3):
                    tpart(co, qs, first=False, last=True)
        tflush((0, 1, 2, 3))
        for co, tg in ((4, "x0"), (5, "x1"), (6, "a0"), (7, "a1")):
            pool = psA if tg.startswith("a") else psX
            tailb[co] = pool.tile([128, 512], f32, name="tb", tag=tg)
        for co in (4, 5, 6, 7):
            for qs in range(4):
                tpart(co, qs, first=(qs == 0), last=(qs == 3))
        tflush((4, 5, 6, 7))
    nc.compile()
    return nc


_PROGRAM = None


def _get_program():
    global _PROGRAM
    if _PROGRAM is None:
        _PROGRAM = _build_program()
    return _PROGRAM


def _pack_rows(w, kt):
    # [kt*128, F] -> [128, kt*F]: partition p holds rows p, 128+p, ...
    F = w.shape[1]
    return np.ascontiguousarray(
        w.reshape(kt, 128, F).transpose(1, 0, 2).reshape(128, kt * F))


def _pack_jo(w):
    # [KT*128, 2*128] -> [128, 2, KT, 128]: jo-major so the jo1 half can
    # load after the x stream
    return np.ascontiguousarray(
        w.reshape(KT, 128, 2, 128).transpose(1, 2, 0, 3).reshape(128, -1))


def _make_in_maps(x, distance_matrix, W_qkv, W_proj):
    ident = np.eye(128, dtype=np.float16)
    in_maps = []
    for core in range(NCORES):
        b, hg = divmod(core, HG)
        sl = slice(hg * DG, (hg + 1) * DG)
        in_maps.append({
            "xT": np.ascontiguousarray(x[b].T).astype(np.float16),
            "wq": _pack_jo(W_qkv[:, sl].astype(np.float16)),
            "wk": _pack_jo(W_qkv[:, C + hg * DG:C + (hg + 1) * DG].astype(np.float16)),
            "wv": _pack_rows(W_qkv[:, 2 * C + hg * DG:2 * C + (hg + 1) * DG].astype(np.float16), KT),
            "wp": _pack_rows(W_proj[sl, :].astype(np.float16), 2),
            "dmt": _pack_rows((0.5 * distance_matrix[b, 0].T).astype(np.float16), MT),
            "ident": ident,
        })
    return in_maps


def kernel(x, distance_matrix, W_qkv, W_proj, b_proj, _results_hook=None):
    from concourse.bass_utils import run_bass_kernel_spmd

    x = np.asarray(x)
    distance_matrix = np.asarray(distance_matrix)
    W_qkv = np.asarray(W_qkv)
    W_proj = np.asarray(W_proj)
    b_proj = np.asarray(b_proj)
    nc = _get_program()
    in_maps = _make_in_maps(x, distance_matrix, W_qkv, W_proj)
    res = run_bass_kernel_spmd(nc, in_maps, list(range(NCORES)))
    if _results_hook is not None:
        _results_hook(res)
    out = np.zeros((B, N, C), dtype=np.float32)
    for core in range(NCORES):
        b = core // HG
        out[b] += res.results[core]["pout"].T
    out += b_proj[None, None, :].astype(np.float32)
    return out
